# revision 1
# baseline (speedup 1.0000x reference)
"""CIEDE2000 loss kernel for Trainium2, 8 NeuronCores, batch-sharded.

Self-contained: takes full inputs img1/img2 [16,3,512,512] f32, returns
full output [16,512,512] f32 (= deltaE_ciede2000(lab(img1), lab(img2))/100).

Strategy: purely elementwise per-pixel -> shard batch over 8 cores (2 each).
Per core, pixels are processed as [128, F] tiles in 4096/F chunks, stage-major
in 5 passes grouped by ACT table set (natural_log_exp / trig alternating), with
cross-pass intermediates spilled to DRAM. pow/sqrt/div chains run on ScalarE
via Ln/Exp with free scale+bias; selects/wraps/reciprocals run on VectorE via
runtime-registered fused custom DVE ops.
"""
import sys

sys.path.insert(0, "/opt/trn_rl_repo")

import numpy as np

import concourse.mybir as mybir
from concourse import dve_ops
from concourse.dve_spec import (
    Spec, Src0, Src1, C0, C1, C2, Zero, One, MaxNeg,
    relu, sq, maxx, minn, select, eq, ne, lower, AluOp, Bin,
    _has_src1,
)
from concourse.dve_uop import DveOpSpec

A = mybir.ActivationFunctionType
ALU = mybir.AluOpType
F32 = mybir.dt.float32
PI = float(np.pi)
K25 = 6103515625.0  # 25**7

N_CORES = 8
B_FULL = 16
B_CORE = B_FULL // N_CORES  # 2 batches per core
H = W = 512
COLS_PER_BATCH = (H * W) // 128  # 2048
COLS = B_CORE * COLS_PER_BATCH  # 4096
F = 512  # chunk free-dim
N_CHUNKS = COLS // F


# --- runtime custom-DVE op registration ------------------------------------
def _register_dve_op(name, spec, subdim=False):
    for op in dve_ops.OPS:
        if op.name == name:
            return op
    row = dve_ops._CUSTOM_DVE_ROW_BASE + len(dve_ops.OPS)
    assert row < 0x20, f"row {row} out of 5-bit range"
    shas = {}
    for ver in ("v3",):
        tmp = DveOpSpec(
            name=name, opcode=row, uops=lower(spec, ver=ver), rd1_en=_has_src1(spec)
        )
        shas[ver] = tmp.sha(ver)
    op = dve_ops.DveOp(name, spec, subdim=subdim, uops_sha=shas)
    dve_ops.OPS.append(op)
    dve_ops.CUSTOM_DVE_SPECS[name] = spec
    dve_ops._SUB_OPCODE_FOR_NAME[name] = row
    return op


SEL_GT_AFFINE = _register_dve_op(
    "SEL_GT_AFFINE",
    Spec(
        body=select(Src0 > C0, Src1, Src0 * C1 + C2),
        reference=lambda in0, in1, s0, s1, imm2: np.where(
            in0 > s0, in1, in0 * s1 + imm2
        ).astype(np.float32),
    ),
)
LIN2B = _register_dve_op(
    "LIN2B",
    Spec(
        body=Src0 * C0 + Src1 * C1 + C2,
        reference=lambda in0, in1, s0, s1, imm2: (
            in0 * s0 + in1 * s1 + imm2
        ).astype(np.float32),
    ),
)
SCALED_SUMSQ = _register_dve_op(
    "SCALED_SUMSQ",
    Spec(
        body=sq(Src0 * C0) + sq(Src1 * C1),
        reference=lambda in0, in1, s0, s1, imm2: (
            (in0 * s0) ** 2 + (in1 * s1) ** 2
        ).astype(np.float32),
    ),
)
MUL2SC = _register_dve_op(
    "MUL2SC",
    Spec(
        body=Src0 * Src1 * C0 + C1,
        reference=lambda in0, in1, s0, s1, imm2: (in0 * in1 * s0 + s1).astype(
            np.float32
        ),
    ),
)
SQ_ADD = _register_dve_op(
    "SQ_ADD",
    Spec(
        body=sq(Src0) + Src1,
        reference=lambda in0, in1, s0, s1, imm2: (in0 * in0 + in1).astype(np.float32),
    ),
)
# atan2 quadrant fix + fold to [0,2pi) + neuron atan2(y,0)=+pi/2 convention:
# hq = at + s0*(a<0); h = hq + s1*(hq<0); out = h - s0*(at < imm2)
# imm2 ~ -(pi/2)+eps detects at==-pi/2-exact (only possible when a was +-0
# and t=b/eps saturated Arctan) -> 3pi/2 - pi = pi/2 as neuron-jax returns.
_hq = Src0 + C0 * ((Src1 < Zero) - (Src0 < C2))
ATAN2_FIX = _register_dve_op(
    "ATAN2_FIX2",
    Spec(
        body=_hq + C1 * (_hq < Zero),
        reference=lambda in0, in1, s0, s1, imm2: (
            lambda hq: (hq + s1 * (hq < 0)).astype(np.float32)
        )(in0 + s0 * ((in1 < 0).astype(np.float32) - (in0 < imm2))),
    ),
)
_absd = maxx(Src1, Zero - Src1)
_m_hb = C0 < _absd
_p_hb = Src0 < C1
HBAR_ADJUST = _register_dve_op(
    "HBAR_ADJUST",
    Spec(
        body=Src0 + _m_hb * (_p_hb * C2 - C1),
        reference=lambda in0, in1, s0, s1, imm2: (
            in0 + (np.abs(in1) > s0) * ((in0 < s1).astype(np.float32) * imm2 - s1)
        ).astype(np.float32),
    ),
)
_y_arw2 = Src0 * C0 + C1
_y2_arw2 = _y_arw2 + _y_arw2
AFF_RANGE_WRAP = _register_dve_op(
    "AFF_RANGE_WRAP",
    Spec(
        body=_y_arw2 + C2 * ((_y2_arw2 < (Zero - C2)) - (C2 < _y2_arw2)),
        reference=lambda in0, in1, s0, s1, imm2: (
            (in0 * s0 + s1)
            + imm2
            * (
                (2 * (in0 * s0 + s1) < -imm2).astype(np.float32)
                - (2 * (in0 * s0 + s1) > imm2).astype(np.float32)
            )
        ).astype(np.float32),
    ),
)


def _patch_act_tables(keep=("natural_log_exp_and_others", "trig_and_small")):
    import functools

    import concourse.hw_specs as hw_specs

    if getattr(hw_specs, "_act_tables_patched", None) == keep:
        return
    orig = hw_specs.get_activation_tables.__wrapped__

    @functools.cache
    def patched(module_arch):
        tables = dict(orig(module_arch))
        return {k: (v if k in keep else set()) for k, v in tables.items()}

    hw_specs.get_activation_tables = patched
    hw_specs._act_tables_patched = keep
    import concourse.bacc as bacc_mod

    bacc_mod.get_activation_tables = patched
    import concourse.bass_interp as bi

    if hasattr(bi, "get_activation_tables"):
        bi.get_activation_tables = patched


def _reg_consts(nc, vals, dtype=mybir.dt.float32):
    new = False
    for val in vals:
        key = (dtype, float(val))
        if key in nc.const_aps.aps:
            continue
        t = nc.alloc_sbuf_tensor(f"const-{dtype.name}-{float(val)}", [128, 1], dtype)
        nc.gpsimd.memset(t.ap(), float(val))
        nc.const_aps.aps[key] = t.ap()
        new = True
    if new:
        nc.all_engine_barrier()


# --- kernel build ----------------------------------------------------------
def _build():
    _patch_act_tables()
    import concourse.bacc as bacc
    from concourse import tile
    from concourse.tile_rust import add_dep_helper

    nc = bacc.Bacc(None, target_bir_lowering=False)
    _reg_consts(
        nc,
        [0.055 / 1.055, K25, 20.0, -66.0, -3.8, -4.605170185988091],
    )
    img1 = nc.dram_tensor("img1", [B_CORE, 3, H, W], F32, kind="ExternalInput")
    img2 = nc.dram_tensor("img2", [B_CORE, 3, H, W], F32, kind="ExternalInput")
    out = nc.dram_tensor("out", [B_CORE, H, W], F32, kind="ExternalOutput")

    # [b, 128, c, 2048] views (one 3-channel DMA per image per chunk)
    v1 = img1.ap().rearrange("b c (p x) w -> b p c (x w)", p=128)
    v2 = img2.ap().rearrange("b c (p x) w -> b p c (x w)", p=128)
    vo = out.ap().rearrange("b (p x) w -> b p (x w)", p=128)

    cnt = [0]
    cur_pass_acts = []
    prev_marker = [None]

    with tile.TileContext(nc) as tc:
        with tc.tile_pool(name="wp", bufs=1) as wp, \
             tc.tile_pool(name="dp", bufs=1, space="DRAM") as dp:

            def chain(bi_):
                # act belongs to current pass; ordered after previous pass's
                # cut marker (table-set grouping without full serialization)
                if prev_marker[0] is not None:
                    add_dep_helper(
                        bi_.ins, prev_marker[0], sync=False, reason="pass-cut"
                    )
                cur_pass_acts.append(bi_)

            def pass_cut():
                # marker nop on ACT engine: depends on all acts of the pass
                mk = wp.tile([128, 1], F32, tag="mark",
                             name=f"mark_{cnt[0]}", bufs=2)
                cnt[0] += 1
                m = nc.scalar.activation(
                    mk[:], nc.const_aps.tensor(0.0, (128, 1)), A.Copy
                )
                for a in cur_pass_acts:
                    add_dep_helper(m.ins, a.ins, sync=False, reason="pass-cut-in")
                cur_pass_acts.clear()
                prev_marker[0] = m.ins

            import collections

            class TagPool:
                def __init__(self, prefix, n, bufs=1):
                    self.avail = collections.deque(
                        f"{prefix}{i}" for i in range(n)
                    )
                    self.bufs = bufs

                def get(self):
                    return self.avail.popleft()

                def put(self, tag):
                    self.avail.append(tag)

            class Val:
                """A [128,F] tile with an owned tag slot; free() returns the
                tag to its pool (call after last use)."""

                def __init__(self, pool, width=None):
                    self.pool = pool
                    self.tag = pool.get()
                    cnt[0] += 1
                    self.tile = wp.tile(
                        [128, width or F], F32, tag=self.tag,
                        name=f"{self.tag}_{cnt[0]}", bufs=pool.bufs,
                    )

                def __getitem__(self, sl):
                    return self.tile[sl]

                def free(self):
                    if self.tag is not None:
                        self.pool.put(self.tag)
                        self.tag = None

            def _ap(x):
                return x[:] if isinstance(x, Val) else x

            def ACT(pool, src, func, scale=1.0, bias=0.0):
                v = Val(pool)
                i = nc.scalar.activation(v[:], _ap(src), func, bias=bias, scale=scale)
                chain(i)
                return v

            def CUST(pool, op, in0, in1=None, s0=0.0, s1=0.0, imm2=0.0):
                v = Val(pool)
                nc.vector._custom_dve(
                    op, out=v[:], in0=_ap(in0),
                    in1=None if in1 is None else _ap(in1),
                    s0=s0, s1=s1, imm2=imm2,
                )
                return v

            def TT(pool, a, b, op):
                v = Val(pool)
                nc.vector.tensor_tensor(v[:], _ap(a), _ap(b), op)
                return v

            def STT(pool, in0, scalar, in1, op0, op1):
                v = Val(pool)
                nc.vector.scalar_tensor_tensor(
                    out=v[:], in0=in0[:], scalar=scalar, in1=in1[:], op0=op0, op1=op1
                )
                return v

            def TS(pool, in0, s1, s2, op0, op1):
                v = Val(pool)
                nc.vector.tensor_scalar(
                    out=v[:], in0=in0[:], scalar1=s1, scalar2=s2, op0=op0, op1=op1
                )
                return v

            def RECIP(pool, x):
                v = Val(pool)
                nc.vector.reciprocal_approx_fast(out=v[:], in_=x[:])
                return v

            planes = {
                n: dp.tile([128, COLS], F32, tag=f"pl_{n}", name=f"pl_{n}")
                for n in ["t1", "t2", "Lt", "Ct", "sCC", "CbS", "a1p", "a2p",
                          "Ht", "u2t", "ee", "s2d"]
            }

            def spill(name, c, v):
                nc.sync.dma_start(
                    out=planes[name][:, c * F:(c + 1) * F], in_=v[:]
                )

            def load(pool, name, c):
                v = Val(pool)
                nc.sync.dma_start(
                    out=v[:], in_=planes[name][:, c * F:(c + 1) * F]
                )
                return v

            def chunk_src(view, c, ch):
                b = c // (COLS_PER_BATCH // F)
                o = (c % (COLS_PER_BATCH // F)) * F
                return view[b, ch][:, o:o + F]

            pa = TagPool("ka", 44, bufs=1)
            pin = TagPool("kv", 2, bufs=2)   # input DMA tiles, double-buffered
            pb = TagPool("kb", 26, bufs=1)
            pe = TagPool("ke", 16, bufs=1)

            # ---------------- PASS A (natural_log_exp) --------------------
            def pass_A(c):
                dd = {}
                for i, view in ((1, v1), (2, v2)):
                    b = c // (COLS_PER_BATCH // F)
                    o = (c % (COLS_PER_BATCH // F)) * F
                    wt = Val(pin, width=3 * F)
                    nc.sync.dma_start(
                        out=wt.tile.rearrange("p (ch f) -> p ch f", ch=3),
                        in_=view[b][:, :, o:o + F],
                    )
                    lins = []
                    for ch in range(3):
                        vts = wt.tile[:, ch * F:(ch + 1) * F]
                        lv = ACT(pa, vts, A.Ln, 1 / 1.055, 0.055 / 1.055)
                        e4 = ACT(pa, lv, A.Exp, 0.4)
                        lv.free()
                        x2 = ACT(pa, vts, A.Square, 1 / 1.055, 0.055 / 1.055)
                        p = TT(pa, x2, e4, ALU.mult)
                        x2.free()
                        e4.free()
                        lin = CUST(pa, SEL_GT_AFFINE, vts, p, 0.04045, 1 / 12.92, 0.0)
                        p.free()
                        lins.append(lin)
                    wt.free()
                    rows = [
                        (0.412453 / 0.95047, 0.357580 / 0.95047, 0.180423 / 0.95047),
                        (0.212671, 0.715160, 0.072169),
                        (0.019334 / 1.08883, 0.119193 / 1.08883, 0.950227 / 1.08883),
                    ]
                    fs = []
                    for r, (ca, cb_, cc) in enumerate(rows):
                        xm = CUST(pa, LIN2B, lins[0], lins[1], ca, cb_, 0.0)
                        xr = Val(pa)
                        nc.vector.affine_then_add(
                            out=xr[:], in0=lins[2][:], in1=xm[:], scale=cc, bias=0.0
                        )
                        xm.free()
                        lt = ACT(pa, xr, A.Ln)
                        cb0 = ACT(pa, lt, A.Exp, 1 / 3)
                        lt.free()
                        # Newton step y=(2*y0 + t/y0^2)/3 tightens the ACT
                        # Exp/Ln cube root from ~3e-5 to ~3e-6 rel (keeps the
                        # |h_diff|~pi select aligned with the reference).
                        yy2 = ACT(pa, cb0, A.Square)
                        rr2 = RECIP(pa, yy2)
                        yy2.free()
                        mm2 = TT(pa, xr, rr2, ALU.mult)
                        rr2.free()
                        cbr = CUST(pa, LIN2B, cb0, mm2, 2.0 / 3.0, 1.0 / 3.0, 0.0)
                        cb0.free()
                        mm2.free()
                        fr = CUST(pa, SEL_GT_AFFINE, xr, cbr, 0.008856, 7.787,
                                  4.0 / 29.0)
                        xr.free()
                        cbr.free()
                        fs.append(fr)
                    for ln_ in lins:
                        ln_.free()
                    d1 = TT(pa, fs[0], fs[1], ALU.subtract)
                    d2 = TT(pa, fs[1], fs[2], ALU.subtract)
                    fs[0].free()
                    fs[2].free()
                    dd[f"d1_{i}"], dd[f"d2_{i}"], dd[f"fy{i}"] = d1, d2, fs[1]

                S1 = CUST(pa, SCALED_SUMSQ, dd["d1_1"], dd["d2_1"], 500.0, 200.0)
                S2 = CUST(pa, SCALED_SUMSQ, dd["d1_2"], dd["d2_2"], 500.0, 200.0)
                lnS1g = ACT(pa, S1, A.Ln)
                lnS2g = ACT(pa, S2, A.Ln)
                S1.free(); S2.free()
                C1 = ACT(pa, lnS1g, A.Exp, 0.5)
                C2 = ACT(pa, lnS2g, A.Exp, 0.5)
                lnS1g.free(); lnS2g.free()
                CbarS = TT(pa, C1, C2, ALU.add)
                C1.free(); C2.free()
                lnCb = ACT(pa, CbarS, A.Ln, 0.5)
                CbarS.free()
                c7 = ACT(pa, lnCb, A.Exp, 7.0)
                lnden = ACT(pa, c7, A.Ln, 1.0, K25)
                garg = CUST(pa, LIN2B, lnCb, lnden, 3.5, -0.5, 0.0)
                lnCb.free(); lnden.free()
                gs0 = ACT(pa, garg, A.Exp)
                garg.free()
                # Newton step: gs = 0.5*(gs0 + R/gs0), R = c7/(c7+K25);
                # tightens ACT-table error (~3e-5) to ~2e-6 so the
                # |h_diff|>pi boundary matches the reference.
                den = TS(pa, c7, 1.0, K25, ALU.mult, ALU.add)
                rden = RECIP(pa, den)
                den.free()
                Rv = TT(pa, c7, rden, ALU.mult)
                rden.free(); c7.free()
                rgs = RECIP(pa, gs0)
                corr = TT(pa, Rv, rgs, ALU.mult)
                Rv.free(); rgs.free()
                gs = CUST(pa, LIN2B, gs0, corr, 0.5, 0.5, 0.0)
                gs0.free(); corr.free()
                scl = ACT(pa, gs, A.Copy, -0.5, 1.5)  # 1+G
                gs.free()
                a1p = CUST(pa, MUL2SC, dd["d1_1"], scl, 500.0, 1e-30)
                a2p = CUST(pa, MUL2SC, dd["d1_2"], scl, 500.0, 1e-30)
                scl.free()
                dd["d1_1"].free(); dd["d1_2"].free()
                spill("a1p", c, a1p)
                spill("a2p", c, a2p)
                Sp1 = CUST(pa, SCALED_SUMSQ, a1p, dd["d2_1"], 1.0, 200.0)
                Sp2 = CUST(pa, SCALED_SUMSQ, a2p, dd["d2_2"], 1.0, 200.0)
                ra1 = RECIP(pa, a1p)
                ra2 = RECIP(pa, a2p)
                a1p.free(); a2p.free()
                t1 = CUST(pa, MUL2SC, dd["d2_1"], ra1, 200.0, 0.0)
                t2 = CUST(pa, MUL2SC, dd["d2_2"], ra2, 200.0, 0.0)
                ra1.free(); ra2.free()
                dd["d2_1"].free(); dd["d2_2"].free()
                spill("t1", c, t1)
                spill("t2", c, t2)
                t1.free(); t2.free()
                lnS1 = ACT(pa, Sp1, A.Ln)
                lnS2 = ACT(pa, Sp2, A.Ln)
                Sp1.free(); Sp2.free()
                C1p = ACT(pa, lnS1, A.Exp, 0.5)
                C2p = ACT(pa, lnS2, A.Exp, 0.5)
                lnSs = TT(pa, lnS1, lnS2, ALU.add)
                lnS1.free(); lnS2.free()
                sCC = ACT(pa, lnSs, A.Exp, 0.25)
                lnSs.free()
                spill("sCC", c, sCC)
                sCC.free()
                CbS = TT(pa, C1p, C2p, ALU.add)
                spill("CbS", c, CbS)
                SC = ACT(pa, CbS, A.Copy, 0.0225, 1.0)
                CbS.free()
                rSC = RECIP(pa, SC)
                SC.free()
                dCp = TT(pa, C2p, C1p, ALU.subtract)
                C1p.free(); C2p.free()
                Ct = TT(pa, dCp, rSC, ALU.mult)
                dCp.free(); rSC.free()
                spill("Ct", c, Ct)
                Ct.free()
                ssum = TT(pa, dd["fy1"], dd["fy2"], ALU.add)
                q2 = ACT(pa, ssum, A.Square, 58.0, -66.0)
                ssum.free()
                lnq = ACT(pa, q2, A.Ln, 1.0, 20.0)
                rsq = ACT(pa, lnq, A.Exp, -0.5)
                lnq.free()
                SL = CUST(pa, MUL2SC, q2, rsq, 0.015, 1.0)
                q2.free(); rsq.free()
                rSL = RECIP(pa, SL)
                SL.free()
                dL = TT(pa, dd["fy2"], dd["fy1"], ALU.subtract)
                dd["fy1"].free(); dd["fy2"].free()
                Lt = CUST(pa, MUL2SC, dL, rSL, 116.0, 0.0)
                dL.free(); rSL.free()
                spill("Lt", c, Lt)
                Lt.free()

            # ---------------- PASS B (trig) -------------------------------
            def pass_B(c):
                t1 = load(pb, "t1", c)
                t2 = load(pb, "t2", c)
                a1p = load(pb, "a1p", c)
                a2p = load(pb, "a2p", c)
                atA = ACT(pb, t1, A.Arctan)
                atB = ACT(pb, t2, A.Arctan)
                t1.free(); t2.free()
                h1 = CUST(pb, ATAN2_FIX, atA, a1p, PI, 2 * PI, -1.5707960)
                h2 = CUST(pb, ATAN2_FIX, atB, a2p, PI, 2 * PI, -1.5707960)
                atA.free(); atB.free(); a1p.free(); a2p.free()
                hd = TT(pb, h2, h1, ALU.subtract)
                hs = TT(pb, h1, h2, ALU.add)
                h1.free(); h2.free()
                Hb2 = CUST(pb, HBAR_ADJUST, hs, hd, PI, 2 * PI, 4 * PI)
                hs.free()
                dH = CUST(pb, AFF_RANGE_WRAP, hd, None, 1.0, 0.0, 2 * PI)
                hd.free()
                sdH = ACT(pb, dH, A.Sin, 0.5)
                dH.free()
                mc = ACT(pb, Hb2, A.Copy, 1.0, -2 * PI)
                Hb2.free()
                y1 = CUST(pb, AFF_RANGE_WRAP, mc, None, 0.5, -2 * PI / 3, 2 * PI)
                s1c = ACT(pb, y1, A.Sin)
                y1.free()
                y2 = CUST(pb, AFF_RANGE_WRAP, mc, None, 1.0, PI / 2, 2 * PI)
                s2c = ACT(pb, y2, A.Sin)
                y3a = CUST(pb, AFF_RANGE_WRAP, mc, None, 1.5, -1.4660766, 2 * PI)
                y3 = CUST(pb, AFF_RANGE_WRAP, y3a, None, 1.0, 0.0, 2 * PI)
                y3a.free()
                s3c = ACT(pb, y3, A.Sin)
                y3.free()
                # 2*y2 - 2.6703537 === 4*Hbar - 63pi/180 + pi/2 (mod 2pi),
                # range (-8.95, 3.61] -> single wrap suffices
                y4 = CUST(pb, AFF_RANGE_WRAP, y2, None, 2.0, -2.6703537, 2 * PI)
                s4c = ACT(pb, y4, A.Sin)
                y4.free()
                y2.free()
                u2t = ACT(pb, mc, A.Square, 1.14591559, -3.8)
                mc.free()
                spill("u2t", c, u2t)
                u2t.free()
                Tt1 = CUST(pb, LIN2B, s1c, s2c, -0.17, 0.24, 1.0)
                s1c.free(); s2c.free()
                Tt2 = STT(pb, s3c, 0.32, Tt1, ALU.mult, ALU.add)
                s3c.free(); Tt1.free()
                T = STT(pb, s4c, -0.20, Tt2, ALU.mult, ALU.add)
                s4c.free(); Tt2.free()
                CbS = load(pb, "CbS", c)
                SH = CUST(pb, MUL2SC, T, CbS, 0.0075, 1.0)
                T.free(); CbS.free()
                rSH = RECIP(pb, SH)
                SH.free()
                sCC = load(pb, "sCC", c)
                dHt = TT(pb, sCC, sdH, ALU.mult)
                sCC.free(); sdH.free()
                Ht = CUST(pb, MUL2SC, dHt, rSH, 2.0, 0.0)
                dHt.free(); rSH.free()
                spill("Ht", c, Ht)
                Ht.free()

            # ---------------- PASS C / D ----------------------------------
            def pass_C(c):
                u2t = load(pe, "u2t", c)
                ee = ACT(pe, u2t, A.Exp, -1.0)
                u2t.free()
                spill("ee", c, ee)
                ee.free()

            def pass_D(c):
                ee = load(pe, "ee", c)
                s2d = ACT(pe, ee, A.Sin, PI / 3)
                ee.free()
                spill("s2d", c, s2d)
                s2d.free()

            # ---------------- PASS E (natural_log_exp) --------------------
            def pass_E(c):
                CbS = load(pe, "CbS", c)
                lnCbp = ACT(pe, CbS, A.Ln, 0.5)
                CbS.free()
                c7p = ACT(pe, lnCbp, A.Exp, 7.0)
                lnden3 = ACT(pe, c7p, A.Ln, 1.0, K25)
                c7p.free()
                rcg = CUST(pe, LIN2B, lnCbp, lnden3, 3.5, -0.5,
                           0.6931471805599453)
                lnCbp.free(); lnden3.free()
                Rc = ACT(pe, rcg, A.Exp)
                rcg.free()
                s2d = load(pe, "s2d", c)
                w = TT(pe, s2d, Rc, ALU.mult)
                s2d.free(); Rc.free()
                Ct = load(pe, "Ct", c)
                Ht = load(pe, "Ht", c)
                Pch = TT(pe, Ct, Ht, ALU.mult)
                Lt = load(pe, "Lt", c)
                Sq1 = CUST(pe, SCALED_SUMSQ, Lt, Ct, 1.0, 1.0)
                Lt.free(); Ct.free()
                Sq2 = CUST(pe, SQ_ADD, Ht, Sq1)
                Ht.free(); Sq1.free()
                nwp = STT(pe, w, -1.0, Pch, ALU.mult, ALU.mult)
                w.free(); Pch.free()
                dE2 = TT(pe, Sq2, nwp, ALU.add)
                Sq2.free(); nwp.free()
                rdE = ACT(pe, dE2, A.Relu)
                dE2.free()
                lnE = ACT(pe, rdE, A.Ln)
                rdE.free()
                ov = ACT(pe, lnE, A.Exp, 0.5, -4.605170185988091)
                lnE.free()
                b = c // (COLS_PER_BATCH // F)
                o = (c % (COLS_PER_BATCH // F)) * F
                nc.sync.dma_start(out=vo[b][:, o:o + F], in_=ov[:])
                ov.free()

            group = N_CHUNKS
            for g0 in range(0, N_CHUNKS, group):
                cs = range(g0, min(g0 + group, N_CHUNKS))
                for c in cs:
                    pass_A(c)
                pass_cut()
                for c in cs:
                    pass_B(c)
                pass_cut()
                for c in cs:
                    pass_C(c)
                pass_cut()
                for c in cs:
                    pass_D(c)
                pass_cut()
                for c in cs:
                    pass_E(c)
                if g0 + group < N_CHUNKS:
                    pass_cut()

    nc.compile()
    return nc


_NC = None


def kernel(img1, img2):
    global _NC
    from concourse.bass_utils import run_bass_kernel_spmd

    img1 = np.ascontiguousarray(np.asarray(img1, dtype=np.float32))
    img2 = np.ascontiguousarray(np.asarray(img2, dtype=np.float32))
    if _NC is None:
        _NC = _build()
    in_maps = [
        {
            "img1": img1[i * B_CORE:(i + 1) * B_CORE],
            "img2": img2[i * B_CORE:(i + 1) * B_CORE],
        }
        for i in range(N_CORES)
    ]
    res = run_bass_kernel_spmd(_NC, in_maps, core_ids=list(range(N_CORES)))
    return np.concatenate([res.results[i]["out"] for i in range(N_CORES)], axis=0)

